# revision 45
# baseline (speedup 1.0000x reference)
"""Trainium2 Bass kernel for a dense transformer block (pre-LN, MHA + GELU MLP).

Sharding: 8 cores = 2 batches x 4 sequence-quarters. Each core recomputes
LN1 + K/V for its full batch (zero cross-core communication), and computes
Q/attention/proj/MLP for its own 512 tokens only.

Device works feature-major ([feature, token]); the host pre-transposes x and
post-transposes the output. LN gains/biases are folded into the following
matmul weights on the host; the v bias is folded into b_proj.

Numerics: all dense GEMMs (qkv, proj, fc1, fc2) run in fp8e4 with DoubleRow
perf mode (~1.4x tensor throughput; weights scaled by SW=32 into e4m3 range
and unscaled at psum evac). QK and AV stay bf16. Softmax exp: even key-tiles
use the exact scalar-engine Exp; odd key-tiles use a Schraudolph-style
bitcast exp approximation on the vector engine (|rel err| < 3%), halving the
scalar-engine load. The residual stream (x2, out), layernorm statistics and
softmax denominators stay fp32; the x residual is bf16.
"""
import sys

sys.path.insert(0, "/opt/trn_rl_repo")

import numpy as np
import ml_dtypes

import concourse.bass as bass  # noqa: F401
import concourse.tile as tile
from concourse import bacc, mybir, bass_utils

F32 = mybir.dt.float32
F32R = mybir.dt.float32r
BF16 = mybir.dt.bfloat16
FP8 = mybir.dt.float8e4
I16 = mybir.dt.int16
DR = mybir.MatmulPerfMode.DoubleRow
AF = mybir.ActivationFunctionType
ALU = mybir.AluOpType

P = 128
D = 768
NH = 12
DH = 64
DFF = 3072
TB = 2048      # tokens per batch
TO = 512       # tokens owned per core
NJ = D // P    # 6 feature tiles
NJP = NJ // 2  # 3 DoubleRow feature-pair tiles
NT = TB // TO  # 4 token tiles per batch
NTK = TB // P  # 16 key tiles
NMLP = DFF // P    # 24
NMLPP = NMLP // 2  # 12 DoubleRow pairs
EPS = 1e-6
N_CORES = 8
VW = 66        # 64 v cols + 2 ones cols per head
SW = 32.0      # fp8 weight quantization scale

# Schraudolph bitcast exp: exp(x) ~= bf16_bits(int16(x*EA + EB))
EA = 184.664965  # 2^7 / ln 2
EB = 16248.6     # 127*128 - 486411/65536 (min-RMS bias correction)


def R(ap):
    return ap.bitcast(F32R)


def _build():
    nc = bacc.Bacc("TRN2", target_bir_lowering=False, debug=False,
                   num_devices=N_CORES)

    x_fm = nc.dram_tensor("x_fm", [D, TB], BF16, kind="ExternalInput").ap()
    wqkv = nc.dram_tensor("wqkv", [D, 3 * D], FP8, kind="ExternalInput").ap()
    bqk = nc.dram_tensor("bqk", [P, 12], F32, kind="ExternalInput").ap()
    wproj = nc.dram_tensor("wproj", [D, D], FP8, kind="ExternalInput").ap()
    bproj = nc.dram_tensor("bproj", [P, NJ], F32, kind="ExternalInput").ap()
    wfc1 = nc.dram_tensor("wfc1", [D, DFF], FP8, kind="ExternalInput").ap()
    bfc1 = nc.dram_tensor("bfc1", [P, NMLP], F32, kind="ExternalInput").ap()
    wfc2 = nc.dram_tensor("wfc2", [DFF, D], BF16, kind="ExternalInput").ap()
    bfc2 = nc.dram_tensor("bfc2", [P, NJ], F32, kind="ExternalInput").ap()
    out_fm = nc.dram_tensor("out_fm", [D, TO], F32, kind="ExternalOutput").ap()

    with nc.allow_low_precision(reason="fp8/bf16 matmuls are intentional"), \
            tile.TileContext(nc) as tc:
        _emit(tc, nc, x_fm, wqkv, bqk, wproj, bproj, wfc1, bfc1,
              wfc2, bfc2, out_fm)
    nc.compile()
    return nc


def _ln_stats_partial(nc, pools, x_tiles, nt, ones_t, sum_all, t1_all,
                      bf16_in, sq_on_gpsimd):
    """Per-512-token-tile LN stats: squares (DVE/gpsimd split), ones-matmul
    sums, and t1 = D*sumsq - sum^2 written into [2, TB] accumulators.
    The Ln/Exp tail runs once, batched, in _ln_finish."""
    sq_pool, ln_ps = pools
    mk = (lambda ap: ap) if bf16_in else R
    sqdt = BF16 if bf16_in else F32
    sl = slice(nt * TO, (nt + 1) * TO)
    xsq = []
    for j in range(NJ):
        t = sq_pool.tile([P, TO], sqdt, tag=f"xsq{j}", name="xsqt")
        nc.vector.tensor_mul(out=mk(t), in0=x_tiles[j], in1=x_tiles[j])
        xsq.append(t)
    ps_sum = ln_ps.tile([2, TO], F32, tag="lnsum", name="ps_sum")
    ps_sq = ln_ps.tile([2, TO], F32, tag="lnsq", name="ps_sq")
    for j in range(NJ):
        nc.tensor.matmul(ps_sum[:], lhsT=mk(ones_t), rhs=mk(x_tiles[j]),
                         start=(j == 0), stop=(j == NJ - 1))
    for j in range(NJ):
        nc.tensor.matmul(ps_sq[:], lhsT=mk(ones_t), rhs=mk(xsq[j]),
                         start=(j == 0), stop=(j == NJ - 1))
    nc.vector.tensor_copy(out=sum_all[:, sl], in_=ps_sum)
    nc.vector.scalar_tensor_tensor(out=t1_all[:, sl], in0=sum_all[:, sl],
                                   scalar=-1.0, in1=sum_all[:, sl],
                                   op0=ALU.mult, op1=ALU.mult)
    nc.vector.scalar_tensor_tensor(out=t1_all[:, sl], in0=ps_sq,
                                   scalar=float(D), in1=t1_all[:, sl],
                                   op0=ALU.mult, op1=ALU.add)


def _ln_finish(nc, stats, sum_all, t1_all, eps2, n_tok, tag):
    """Batched Ln/Exp tail: rs = 1/sqrt(var+eps), cc = -mean*rs."""
    nc.scalar.activation(out=t1_all, in_=t1_all, func=AF.Ln, bias=eps2,
                         scale=1.0 / (D * D))
    rs = stats.tile([2, n_tok], F32, tag=f"rs{tag}", name="rs")
    nc.scalar.activation(out=R(rs), in_=t1_all, func=AF.Exp, scale=-0.5)
    cc = stats.tile([2, n_tok], F32, tag=f"cc{tag}", name="cc")
    nc.vector.scalar_tensor_tensor(out=R(cc), in0=sum_all, scalar=-1.0 / D,
                                   in1=rs, op0=ALU.mult, op1=ALU.mult)
    return rs, cc


def _ln_apply(nc, pools, x_tiles, rs, cc, half2, out_fn, mul_on_gpsimd):
    """Broadcast rs/cc to [128, 512]; out_fn(j) = x*rs + cc (mul on gpsimd,
    add on DVE)."""
    sq_pool, bc_ps = pools
    ps_a = bc_ps.tile([P, TO], F32, tag="bca", name="ps_a")
    nc.tensor.matmul(ps_a[:], lhsT=R(half2), rhs=R(rs), start=True, stop=True)
    ps_c = bc_ps.tile([P, TO], F32, tag="bcc", name="ps_c")
    nc.tensor.matmul(ps_c[:], lhsT=R(half2), rhs=R(cc), start=True, stop=True)
    a_sb = sq_pool.tile([P, TO], BF16, tag="asb", name="a_sb")
    c_sb = sq_pool.tile([P, TO], BF16, tag="csb", name="c_sb")
    nc.vector.tensor_copy(out=a_sb, in_=ps_a)
    nc.vector.tensor_copy(out=c_sb, in_=ps_c)
    for j in range(NJ):
        tmp = sq_pool.tile([P, TO], BF16, tag=f"tmp{j}", name="xnt")
        nc.vector.tensor_mul(out=tmp, in0=x_tiles[j], in1=a_sb)
        nc.vector.tensor_add(out=out_fn(j), in0=tmp, in1=c_sb)


def _emit(tc, nc, x_fm, wqkv, bqk, wproj_d, bproj_d, wfc1_d, bfc1_d,
          wfc2_d, bfc2_d, out_fm):
    ctx_pools = []

    cons_pool = tc.alloc_tile_pool(name="cons", bufs=1)
    ctx_pools.append(cons_pool)
    ones2b = cons_pool.tile([P, 2], BF16)
    nc.vector.memset(ones2b, 1.0)
    ones2 = cons_pool.tile([P, 2], F32)
    nc.vector.memset(ones2, 1.0)
    half2 = cons_pool.tile([2, P], F32)
    nc.vector.memset(half2, 0.5)
    eps2 = cons_pool.tile([2, 1], F32)
    nc.vector.memset(eps2, EPS)

    bqk_sb = cons_pool.tile([P, 12], F32)
    nc.sync.dma_start(out=bqk_sb, in_=bqk)
    bproj_sb = cons_pool.tile([P, NJ], F32)
    nc.sync.dma_start(out=bproj_sb, in_=bproj_d)
    bfc1_sb = cons_pool.tile([P, NMLP], F32)
    nc.sync.dma_start(out=bfc1_sb, in_=bfc1_d)
    bfc2_sb = cons_pool.tile([P, NJ], F32)
    nc.sync.dma_start(out=bfc2_sb, in_=bfc2_d)

    stats = tc.alloc_tile_pool(name="stats", bufs=2)
    ctx_pools.append(stats)

    # k/q bf16; x_own (bf16 residual, doubles as the nt=0 LN input).
    persist = tc.alloc_tile_pool(name="persist", bufs=1)
    k_sb = [persist.tile([P, TB], BF16, tag=f"k{j}", name=f"k{j}")
            for j in range(NJ)]
    q_sb = [persist.tile([P, TO], BF16, tag=f"q{j}", name=f"q{j}")
            for j in range(NJ)]
    x_own = [persist.tile([P, TO], BF16, tag=f"xo{j}", name=f"xo{j}")
             for j in range(NJ)]

    v_pool = tc.alloc_tile_pool(name="vpool", bufs=1, side="right")
    v_sb = [v_pool.tile([P, NH * VW], BF16, tag=f"v{t}", name=f"v{t}")
            for t in range(NTK)]

    # LN1 output: fp8, feature-pair tiles [128, 2, TB] for DoubleRow qkv.
    xn_pool = tc.alloc_tile_pool(name="xnpool", bufs=1)
    xn_all = [xn_pool.tile([P, 2, TB], FP8, tag=f"xn{j}", name=f"xn{j}")
              for j in range(NJP)]

    # ------------- Phase 1: fused load x + LN1 + Q/K/V (fp8 DoubleRow) ------
    with (
        tc.tile_pool(name="xstream", bufs=1) as xpool,
        tc.tile_pool(name="sqpool", bufs=2) as sq_pool,
        tc.tile_pool(name="appool", bufs=1) as ap_pool,
        tc.tile_pool(name="wkq", bufs=1) as wkq_pool,
        tc.tile_pool(name="wv", bufs=1) as wv_pool,
        tc.tile_pool(name="lnps", bufs=1, space="PSUM") as ln_ps,
        tc.tile_pool(name="bcps", bufs=1, space="PSUM") as bc_ps,
        tc.tile_pool(name="mmps", bufs=2, space="PSUM") as mm_ps,
        tc.tile_pool(name="vps5", bufs=2, space="PSUM") as v_ps5,
    ):
        # x DMAs first (the LN chain gates everything); weights after.
        xts = []
        for nt in range(NT):
            if nt == 0:
                xt = x_own
            else:
                xt = [xpool.tile([P, TO], BF16, tag=f"xs{nt}_{j}",
                                 name=f"xs{nt}_{j}") for j in range(NJ)]
            for j in range(NJ):
                nc.sync.dma_start(
                    out=xt[j],
                    in_=x_fm[j * P:(j + 1) * P, nt * TO:(nt + 1) * TO])
            xts.append(xt)
        wkq = []
        for jp in range(NJP):
            t = wkq_pool.tile([P, 2, 2 * D], FP8, tag=f"wkq{jp}",
                              name=f"wkq{jp}")
            for i in range(2):
                r0 = (2 * jp + i) * P
                nc.sync.dma_start(out=t[:, i, :], in_=wqkv[r0:r0 + P, 0:2 * D])
            wkq.append(t)
        wv = []
        for jp in range(NJP):
            t = wv_pool.tile([P, 2, D], FP8, tag=f"wv{jp}", name=f"wv{jp}")
            for i in range(2):
                r0 = (2 * jp + i) * P
                nc.sync.dma_start(out=t[:, i, :],
                                  in_=wqkv[r0:r0 + P, 2 * D:3 * D])
            wv.append(t)

        # LN1 stats batched in halves: stats for nt 0-1, then one Ln + one
        # Exp for those 1024 tokens (fewer act-table loads); the second half
        # runs while qkv matmuls for the first half keep the PE busy.
        sum_all = stats.tile([2, TB], F32, tag="sum1", name="sum_all")
        t1_all = stats.tile([2, TB], F32, tag="t11", name="t1_all")
        rs_all = stats.tile([2, TB], F32, tag="rs1", name="rs_all")
        cc_all = stats.tile([2, TB], F32, tag="cc1", name="cc_all")

        def _emit_qkv_nt(nt):
            tsl = slice(nt * TO, (nt + 1) * TO)
            _ln_apply(nc, (ap_pool, bc_ps), xts[nt], rs_all[:, tsl],
                      cc_all[:, tsl], half2,
                      lambda j, nt=nt: xn_all[j // 2][:, j % 2,
                                                      nt * TO:(nt + 1) * TO],
                      True)
            if nt == 0:
                # Q for own tokens (psum = SW*8*q; evac rescales, adds bias)
                for m in range(NJ):
                    pt = mm_ps.tile([P, TO], F32, tag="mm", name="mmq")
                    for jp in range(NJP):
                        nc.tensor.matmul(
                            pt[:], lhsT=wkq[jp][:, :, m * P:(m + 1) * P],
                            rhs=xn_all[jp][:, :, 0:TO],
                            start=(jp == 0), stop=(jp == NJP - 1),
                            perf_mode=DR)
                    nc.scalar.activation(out=q_sb[m], in_=pt,
                                         func=AF.Identity,
                                         bias=bqk_sb[:, m:m + 1],
                                         scale=1.0 / (8.0 * SW))
            # K for this token tile
            for m in range(NJ):
                pt = mm_ps.tile([P, TO], F32, tag="mm", name="mmk")
                for jp in range(NJP):
                    nc.tensor.matmul(
                        pt[:], lhsT=wkq[jp][:, :, D + m * P:D + (m + 1) * P],
                        rhs=xn_all[jp][:, :, nt * TO:(nt + 1) * TO],
                        start=(jp == 0), stop=(jp == NJP - 1), perf_mode=DR)
                nc.scalar.activation(out=k_sb[m][:, nt * TO:(nt + 1) * TO],
                                     in_=pt, func=AF.Identity,
                                     bias=bqk_sb[:, 6 + m:7 + m],
                                     scale=1.0 / SW)
            # V for this token tile (token-major with ones columns)
            for mt in range(4 * nt, 4 * nt + 4):
                vt = v_sb[mt]
                v3 = vt.rearrange("p (h w) -> p h w", w=VW)
                nc.gpsimd.memset(v3[:, :, 64:66], 1.0)
                pt5 = v_ps5.tile([P, TO], F32, tag="v5", name="v5")
                pt2 = mm_ps.tile([P, TO], F32, tag="mm", name="v2")
                for jp in range(NJP):
                    lhs = xn_all[jp][:, :, mt * P:(mt + 1) * P]
                    nc.tensor.matmul(pt5[:], lhsT=lhs, rhs=wv[jp][:, :, 0:512],
                                     start=(jp == 0), stop=(jp == NJP - 1),
                                     perf_mode=DR)
                for jp in range(NJP):
                    lhs = xn_all[jp][:, :, mt * P:(mt + 1) * P]
                    nc.tensor.matmul(pt2[:, 0:256], lhsT=lhs,
                                     rhs=wv[jp][:, :, 512:768],
                                     start=(jp == 0), stop=(jp == NJP - 1),
                                     perf_mode=DR)
                nc.scalar.activation(
                    out=v3[:, 0:8, 0:64],
                    in_=pt5.rearrange("p (h w) -> p h w", w=64),
                    func=AF.Identity, scale=1.0 / SW)
                nc.scalar.activation(
                    out=v3[:, 8:12, 0:64],
                    in_=pt2[:, 0:256].rearrange("p (h w) -> p h w", w=64),
                    func=AF.Identity, scale=1.0 / SW)

        for half in range(2):
            for nt in (2 * half, 2 * half + 1):
                _ln_stats_partial(nc, (sq_pool, ln_ps), xts[nt], nt, ones2b,
                                  sum_all, t1_all, True, True)
            hsl = slice(half * 2 * TO, (half + 1) * 2 * TO)
            nc.scalar.activation(out=t1_all[:, hsl], in_=t1_all[:, hsl],
                                 func=AF.Ln, bias=eps2, scale=1.0 / (D * D))
            nc.scalar.activation(out=R(rs_all[:, hsl]), in_=t1_all[:, hsl],
                                 func=AF.Exp, scale=-0.5)
            nc.vector.scalar_tensor_tensor(
                out=R(cc_all[:, hsl]), in0=sum_all[:, hsl], scalar=-1.0 / D,
                in1=rs_all[:, hsl], op0=ALU.mult, op1=ALU.mult)
            for nt in (2 * half, 2 * half + 1):
                _emit_qkv_nt(nt)
    xn_pool.release()

    # ---------------- Phase 2: attention (QK/AV bf16, split exp) -----------
    attn_pool = tc.alloc_tile_pool(name="attnpool", bufs=1)
    # fp8 feature-pair tiles [128, 2, TO]: pair jp holds feature blocks
    # 2jp (heads 4jp..) and 2jp+1, ready as DoubleRow rhs for proj.
    attn_fm = [attn_pool.tile([P, 2, TO], FP8, tag=f"at{jp}", name=f"at{jp}")
               for jp in range(NJP)]
    wp_pool = tc.alloc_tile_pool(name="wproj", bufs=1)
    wp = []
    for jp in range(NJP):
        t = wp_pool.tile([P, 2, D], FP8, tag=f"wp{jp}", name=f"wp{jp}")
        for i in range(2):
            r0 = (2 * jp + i) * P
            nc.sync.dma_start(out=t[:, i, :], in_=wproj_d[r0:r0 + P, :])
        wp.append(t)
    with (
        tc.tile_pool(name="seps", bufs=2, space="PSUM") as se_ps,
        tc.tile_pool(name="avps", bufs=2, space="PSUM") as av_ps,
        tc.tile_pool(name="sesb", bufs=4) as se_pool,
        tc.tile_pool(name="divsb", bufs=2) as div_pool,
    ):
        LAG = 2  # AV trails QK by 2 iterations so exp latency is hidden

        def _emit_div(hp, pt_av_a, pt_av_b):
            # normalize: rec = 1/denominator (approx), partition-broadcast
            # on gpsimd, multiply straight out of psum.
            for head, pt_av in ((0, pt_av_a), (1, pt_av_b)):
                rec = div_pool.tile([2, TO], F32, tag="rc", name="rc")
                nc.vector.reciprocal(out=rec[0:1, :], in_=pt_av[64:65, :])
                bc_sb = div_pool.tile([64, TO], F32, tag="bc", name="bcsb")
                nc.gpsimd.partition_broadcast(bc_sb, rec[0:1, :])
                nc.vector.tensor_mul(
                    out=attn_fm[hp // 2][head * 64:(head + 1) * 64,
                                         hp % 2, :],
                    in0=pt_av[0:64, :], in1=bc_sb)

        prev_div = None
        for hp in range(NJ):
            pt_av_a = av_ps.tile([P, TO], F32, tag="ava", name="ava")
            pt_av_b = av_ps.tile([P, TO], F32, tag="avb", name="avb")
            pend = []

            def _flush(last):
                tk, fa, fb = pend.pop(0)
                nc.tensor.matmul(
                    pt_av_a[:VW, :],
                    lhsT=v_sb[tk][:, (2 * hp) * VW:(2 * hp + 1) * VW],
                    rhs=fa, start=(tk == 0), stop=last)
                nc.tensor.matmul(
                    pt_av_b[:VW, :],
                    lhsT=v_sb[tk][:, (2 * hp + 1) * VW:(2 * hp + 2) * VW],
                    rhs=fb, start=(tk == 0), stop=last)

            for tk in range(NTK):
                ksl = slice(tk * P, (tk + 1) * P)
                ps_a = se_ps.tile([P, TO], F32, tag="sea", name="psea")
                ps_b = se_ps.tile([P, TO], F32, tag="seb", name="pseb")
                nc.tensor.matmul(ps_a[:], lhsT=k_sb[hp][0:64, ksl],
                                 rhs=q_sb[hp][0:64, :], start=True, stop=True)
                nc.tensor.matmul(ps_b[:], lhsT=k_sb[hp][64:128, ksl],
                                 rhs=q_sb[hp][64:128, :], start=True,
                                 stop=True)
                # exp: even key-tiles exact (scalar engine), odd key-tiles
                # Schraudolph bitcast approximation (DVE).
                se_a = se_pool.tile([P, TO], BF16, tag="sea", name="sea")
                se_b = se_pool.tile([P, TO], BF16, tag="seb", name="seb")
                for ps, se in ((ps_a, se_a), (ps_b, se_b)):
                    if tk % 2 == 0:
                        nc.scalar.activation(out=se, in_=ps, func=AF.Exp)
                    else:
                        nc.vector.tensor_scalar(se.bitcast(I16), ps, EA, EB,
                                                ALU.mult, ALU.add)
                pend.append((tk, se_a, se_b))
                if len(pend) > LAG:
                    _flush(False)
                if tk == 5 and prev_div is not None:
                    # previous head-pair's normalization, deferred so its
                    # DVE ops don't stall this pair's pipeline start
                    _emit_div(*prev_div)
                    prev_div = None
            while pend:
                _flush(len(pend) == 1)
            prev_div = (hp, pt_av_a, pt_av_b)
        _emit_div(*prev_div)
    v_pool.release()

    # ------------- Phase 3: proj (fp8 DR) + residual -> x2 (fp32) ----------
    x2_pool = tc.alloc_tile_pool(name="x2pool", bufs=1, side="right")
    ctx_pools.append(x2_pool)
    x2_sb = [x2_pool.tile([P, TO], F32, tag=f"x2{j}", name=f"x2{j}")
             for j in range(NJ)]
    wfc1_pool = tc.alloc_tile_pool(name="wfc1", bufs=1, side="right")
    ctx_pools.append(wfc1_pool)
    wf1 = []
    for jp in range(NJP):
        t = wfc1_pool.tile([P, 2, DFF], FP8, tag=f"wf1{jp}", name=f"wf1{jp}")
        for i in range(2):
            r0 = (2 * jp + i) * P
            nc.sync.dma_start(out=t[:, i, :], in_=wfc1_d[r0:r0 + P, :])
        wf1.append(t)
    h_pool = tc.alloc_tile_pool(name="hpool", bufs=1, side="right")
    ctx_pools.append(h_pool)
    h_sb = [h_pool.tile([P, 2, TO], FP8, tag=f"h{jp}", name=f"h{jp}")
            for jp in range(NJP)]
    with (
        tc.tile_pool(name="mmps2", bufs=2, space="PSUM") as mm_ps2,
        tc.tile_pool(name="prsb", bufs=4) as pr_pool,
        tc.tile_pool(name="sqpool2", bufs=2) as sq_pool2,
        tc.tile_pool(name="lnps2", bufs=1, space="PSUM") as ln_ps2,
        tc.tile_pool(name="bcps3", bufs=1, space="PSUM") as bc_ps3,
    ):
        for m in range(NJ):
            pt = mm_ps2.tile([P, TO], F32, tag="mm", name="mmproj")
            for jp in range(NJP):
                nc.tensor.matmul(pt[:], lhsT=wp[jp][:, :, m * P:(m + 1) * P],
                                 rhs=attn_fm[jp],
                                 start=(jp == 0), stop=(jp == NJP - 1),
                                 perf_mode=DR)
            t = pr_pool.tile([P, TO], F32, tag="pr", name="pr")
            nc.vector.tensor_scalar(t, pt, 1.0 / SW, bproj_sb[:, m:m + 1],
                                    ALU.mult, ALU.add)
            nc.vector.tensor_add(out=R(x2_sb[m]), in0=t, in1=x_own[m])
        # LN2 fused into the same block so stats overlap the proj tail
        sum2 = stats.tile([2, TO], F32, tag="sum2", name="sum2")
        t12 = stats.tile([2, TO], F32, tag="t12", name="t12")
        _ln_stats_partial(nc, (sq_pool2, ln_ps2), x2_sb, 0, ones2,
                          sum2, t12, False, False)
        rs2, cc2 = _ln_finish(nc, stats, sum2, t12, eps2, TO, 2)
        _ln_apply(nc, (sq_pool2, bc_ps3), x2_sb, rs2, cc2, half2,
                  lambda j: h_sb[j // 2][:, j % 2, :], False)
    wp_pool.release()
    attn_pool.release()
    persist.release()

    # ---------------- Phase 5: fc1 (fp8 DR) + gelu -> h1 (fp8 pairs) -------
    h1_pool = tc.alloc_tile_pool(name="h1", bufs=1, side="right")
    ctx_pools.append(h1_pool)
    h1_sb = [h1_pool.tile([P, TO], BF16, tag=f"h1{m}", name=f"h1{m}")
             for m in range(NMLP)]
    with (
        tc.tile_pool(name="mmps3", bufs=4, space="PSUM") as mm_ps3,
    ):
        for m in range(NMLP):
            pt = mm_ps3.tile([P, TO], F32, tag="mm", name="mmfc1")
            for jp in range(NJP):
                nc.tensor.matmul(pt[:],
                                 lhsT=wf1[jp][:, :, m * P:(m + 1) * P],
                                 rhs=h_sb[jp],
                                 start=(jp == 0), stop=(jp == NJP - 1),
                                 perf_mode=DR)
            nc.scalar.activation(out=h1_sb[m], in_=pt,
                                 func=AF.Gelu, bias=bfc1_sb[:, m:m + 1],
                                 scale=1.0 / SW)

    # ---------------- Phase 6: fc2 (bf16) + residual + store ---------------
    with (
        tc.tile_pool(name="wfc2", bufs=6) as wfc2_pool,
        tc.tile_pool(name="fc2ps", bufs=1, space="PSUM") as fc2_ps,
        tc.tile_pool(name="outsb", bufs=2) as out_pool,
    ):
        pts = [fc2_ps.tile([P, TO], F32, tag=f"fc2_{m}", name=f"fc2_{m}")
               for m in range(NJ)]
        for j in range(NMLP):
            wt = wfc2_pool.tile([P, D], BF16, tag="wf2", name="wf2")
            nc.sync.dma_start(out=wt, in_=wfc2_d[j * P:(j + 1) * P, :])
            for m in range(NJ):
                nc.tensor.matmul(pts[m][:], lhsT=wt[:, m * P:(m + 1) * P],
                                 rhs=h1_sb[j],
                                 start=(j == 0), stop=(j == NMLP - 1))
        for m in range(NJ):
            ot = out_pool.tile([P, TO], F32, tag="out", name="ot")
            nc.vector.scalar_tensor_tensor(
                out=ot, in0=pts[m], scalar=bfc2_sb[:, m:m + 1],
                in1=x2_sb[m], op0=ALU.add, op1=ALU.add)
            nc.sync.dma_start(out=out_fm[m * P:(m + 1) * P, :], in_=ot)

    for pool in reversed(ctx_pools):
        pool.release()


_NC_CACHE = {}


def _get_nc():
    if "nc" not in _NC_CACHE:
        _NC_CACHE["nc"] = _build()
    return _NC_CACHE["nc"]


def _host_prep(inputs):
    f32 = lambda a: np.ascontiguousarray(np.asarray(a, dtype=np.float32))
    x = f32(inputs["x"])            # [2, 2048, 768]
    W_qkv = f32(inputs["W_qkv"])    # [768, 2304]
    b_qkv = f32(inputs["b_qkv"])
    W_proj = f32(inputs["W_proj"])
    b_proj = f32(inputs["b_proj"])
    W_fc1 = f32(inputs["W_fc1"])
    b_fc1 = f32(inputs["b_fc1"])
    W_fc2 = f32(inputs["W_fc2"])
    b_fc2 = f32(inputs["b_fc2"])
    ln1_g = f32(inputs["ln1_g"])
    ln1_b = f32(inputs["ln1_b"])
    ln2_g = f32(inputs["ln2_g"])
    ln2_b = f32(inputs["ln2_b"])

    scale = DH ** -0.5
    wqkv_eff = W_qkv * ln1_g[:, None]
    bqkv_eff = ln1_b @ W_qkv + b_qkv
    bqkv_eff_q = bqkv_eff[:D] * scale
    bqk = np.concatenate([bqkv_eff_q, bqkv_eff[D:2 * D]]).astype(np.float32)
    bv = bqkv_eff[2 * D:]
    bproj_eff = (b_proj + bv @ W_proj).astype(np.float32)
    wfc1_eff = (W_fc1 * ln2_g[:, None]).astype(np.float32)
    bfc1_eff = (ln2_b @ W_fc1 + b_fc1).astype(np.float32)

    bf = lambda a: np.ascontiguousarray(a.astype(ml_dtypes.bfloat16))
    f8 = lambda a: np.ascontiguousarray(
        np.clip(a * SW, -240.0, 240.0).astype(ml_dtypes.float8_e4m3))
    pack = lambda b: np.ascontiguousarray(
        b.reshape(-1, P).T.astype(np.float32))
    shared = {
        "wqkv": f8(wqkv_eff),
        "bqk": pack(bqk),
        "wproj": f8(W_proj),
        "bproj": pack(bproj_eff),
        "wfc1": f8(wfc1_eff),
        "bfc1": pack(bfc1_eff),
        "wfc2": bf(W_fc2),
        "bfc2": pack(b_fc2),
    }
    in_maps = []
    for c in range(N_CORES):
        b, q = divmod(c, 4)
        xb = np.roll(x[b], -TO * q, axis=0)  # own tokens at rows 0:TO
        m = dict(shared)
        m["x_fm"] = bf(xb.T)
        in_maps.append(m)
    return in_maps


def _run(inputs, trace=False):
    nc = _get_nc()
    in_maps = _host_prep(inputs)
    res = bass_utils.run_bass_kernel_spmd(nc, in_maps, list(range(N_CORES)),
                                          trace=trace)
    B = 2
    out = np.empty((B, TB, D), dtype=np.float32)
    for c in range(N_CORES):
        b, q = divmod(c, 4)
        out[b, TO * q:TO * (q + 1), :] = res.results[c]["out_fm"].T
    return out, res


def kernel(**inputs):
    out, _ = _run(inputs, trace=False)
    return out


if __name__ == "__main__":
    print("building...")
    _get_nc()
    print("built ok")
